# revision 1
# baseline (speedup 1.0000x reference)
"""Trainium2 Bass kernel for nn_DHDN_Dynamic (hypergraph GNN + attention + VAE).

Self-contained: takes FULL inputs as numpy arrays, shards batch over 8
NeuronCores (pure data parallel), runs one Bass/Tile kernel per core,
gathers the full output.
"""
import sys
sys.path.insert(0, '/opt/trn_rl_repo')
import numpy as np

import concourse.bass as bass
from concourse import bacc
import concourse.mybir as mybir
from concourse.tile import TileContext
from concourse.bass_utils import run_bass_kernel_spmd
from concourse.masks import make_identity

F32 = mybir.dt.float32
U32 = mybir.dt.uint32
AF = mybir.ActivationFunctionType
OP = mybir.AluOpType

B, J, H, MAXK, PLAT = 4096, 19, 256, 8, 64
NHEAD, DH = 4, 64
NCORES = 8
BC = B // NCORES          # graphs per core
GPT = 6                   # graphs per (block-diag) tile
RF = GPT * J              # 114 rows per full tile
NEG = -1.0e9

_CACHE = {}


def _tiles(bc):
    """List of (tile_idx, g0, G) covering bc graphs in 6-graph tiles."""
    out = []
    g0 = 0
    t = 0
    while g0 < bc:
        G = min(GPT, bc - g0)
        out.append((t, g0, G))
        g0 += G
        t += 1
    return out


def _chunks(bc):
    """Two chunks of graphs: [(g0, g1, tiles)]"""
    tl = _tiles(bc)
    half = (len(tl) + 1) // 2
    c0 = tl[:half]
    c1 = tl[half:]
    out = []
    for ts_ in (c0, c1):
        if not ts_:
            continue
        g0 = ts_[0][1]
        g1 = ts_[-1][1] + ts_[-1][2]
        out.append((g0, g1, ts_))
    return out


def build_nc(bc=BC, stages=4, sub=9):
    """Build the Bass IR for one core processing bc graphs."""
    nc = bacc.Bacc("TRN2", target_bir_lowering=False)
    R = bc * J  # total rows (tokens)

    # ---------------- DRAM I/O ----------------
    d_pts = nc.dram_tensor("pts", [R, 2], F32, kind="ExternalInput")
    d_feat = nc.dram_tensor("feat", [R, H], F32, kind="ExternalInput")
    d_kvrem = nc.dram_tensor("kvrem", [R, 8], F32, kind="ExternalInput")
    d_rinv = nc.dram_tensor("rinv", [R, 1], F32, kind="ExternalInput")
    d_bo = nc.dram_tensor("bo", [RF, RF], F32, kind="ExternalInput")
    d_bo4 = nc.dram_tensor("bo4", [RF, 512], F32, kind="ExternalInput")
    d_epsT = nc.dram_tensor("epsT", [PLAT, bc + 8], F32, kind="ExternalInput")
    d_wc0 = nc.dram_tensor("wc0", [3, H], F32, kind="ExternalInput")
    d_wga = nc.dram_tensor("wga", [128, 2, H], F32, kind="ExternalInput")   # layers 1,2 half0
    d_wgb = nc.dram_tensor("wgb", [128, 2, H], F32, kind="ExternalInput")   # layers 1,2 half1
    d_cgn = nc.dram_tensor("cgn", [2, H], F32, kind="ExternalInput")
    d_bg3 = nc.dram_tensor("bg3", [3, H], F32, kind="ExternalInput")        # b_gnn rows
    d_wqa = nc.dram_tensor("wqa", [128, 3 * H], F32, kind="ExternalInput")
    d_wqb = nc.dram_tensor("wqb", [128, 3 * H], F32, kind="ExternalInput")
    d_cq = nc.dram_tensor("cq", [1, 3 * H], F32, kind="ExternalInput")
    d_wao4 = nc.dram_tensor("wao4", [64, 4, H], F32, kind="ExternalInput")
    d_cao = nc.dram_tensor("cao", [1, H], F32, kind="ExternalInput")
    d_we1 = nc.dram_tensor("we1", [128, 38, H], F32, kind="ExternalInput")
    d_be1 = nc.dram_tensor("be1", [128, 2], F32, kind="ExternalInput")
    d_we2 = nc.dram_tensor("we2", [128, 2, 2, 64], F32, kind="ExternalInput")
    d_be2 = nc.dram_tensor("be2", [64, 2], F32, kind="ExternalInput")
    d_wdec = nc.dram_tensor("wdec", [PLAT, 38, 128], F32, kind="ExternalInput")
    d_bdec = nc.dram_tensor("bdec", [128, 38], F32, kind="ExternalInput")
    d_wr1 = nc.dram_tensor("wr1", [128, 38, H], F32, kind="ExternalInput")
    d_br1 = nc.dram_tensor("br1", [128, 2], F32, kind="ExternalInput")
    d_wr2 = nc.dram_tensor("wr2", [128, 2, 38], F32, kind="ExternalInput")
    d_br2 = nc.dram_tensor("br2", [38, 1], F32, kind="ExternalInput")
    d_y = nc.dram_tensor("y", [bc, 38], F32, kind="ExternalOutput")

    chunks = _chunks(bc)
    NV = max(g1 - g0 for g0, g1, _ in chunks)  # VAE free dim (padded)
    NV = max(NV, chunks[0][1] - chunks[0][0])

    with TileContext(nc) as tc:
        with tc.tile_pool(name="cst", bufs=1) as cst, \
             tc.tile_pool(name="wk", bufs=4) as wk, \
             tc.tile_pool(name="hbuf", bufs=1) as hbuf, \
             tc.tile_pool(name="io", bufs=3) as io, \
             tc.tile_pool(name="sbA", bufs=3) as sbA, \
             tc.tile_pool(name="sbB", bufs=2) as sbB, \
             tc.tile_pool(name="sbC", bufs=2) as sbC, \
             tc.tile_pool(name="small", bufs=4) as small, \
             tc.tile_pool(name="pm", bufs=3, space="PSUM") as pm, \
             tc.tile_pool(name="pr", bufs=1, space="PSUM") as pr, \
             tc.tile_pool(name="pt", bufs=2, space="PSUM") as pt:

            # ----- constants -----
            ident = cst.tile([128, 128], F32)
            make_identity(nc, ident[:])
            zcol = cst.tile([128, 1], F32)
            nc.vector.memset(zcol[:], 0.0)
            negt = cst.tile([128, 8], F32)
            nc.vector.memset(negt[:], -2.0e9)
            epsc = cst.tile([128, 1], F32)
            nc.vector.memset(epsc[:], 1.0e-5)
            bo_t = cst.tile([RF, RF], F32)
            nc.sync.dma_start(bo_t[:], d_bo[:])
            bo4_t = cst.tile([RF, 512], F32)
            nc.sync.dma_start(bo4_t[:], d_bo4[:])
            wc0_t = cst.tile([3, H], F32)
            nc.sync.dma_start(wc0_t[:], d_wc0[:])
            wga_t = cst.tile([128, 2, H], F32)
            nc.sync.dma_start(wga_t[:], d_wga[:])
            wgb_t = cst.tile([128, 2, H], F32)
            nc.sync.dma_start(wgb_t[:], d_wgb[:])
            cgnb_t = cst.tile([128, 2, H], F32)
            nc.sync.dma_start(cgnb_t[:], d_cgn[None, :, :].to_broadcast([128, 2, H]))
            bg3_t = cst.tile([3, H], F32)
            nc.sync.dma_start(bg3_t[:], d_bg3[:])
            wqa_t = cst.tile([128, 3 * H], F32)
            nc.sync.dma_start(wqa_t[:], d_wqa[:])
            wqb_t = cst.tile([128, 3 * H], F32)
            nc.sync.dma_start(wqb_t[:], d_wqb[:])
            cqb_t = cst.tile([128, 3 * H], F32)
            nc.sync.dma_start(cqb_t[:], d_cq[None, 0, :].to_broadcast([128, 3 * H]))
            wao4_t = cst.tile([64, 4, H], F32)
            nc.sync.dma_start(wao4_t[:], d_wao4[:])
            caob_t = cst.tile([128, H], F32)
            nc.sync.dma_start(caob_t[:], d_cao[None, 0, :].to_broadcast([128, H]))
            we2_t = cst.tile([128, 2, 2, 64], F32)
            nc.sync.dma_start(we2_t[:], d_we2[:])
            be1_t = cst.tile([128, 2], F32)
            nc.sync.dma_start(be1_t[:], d_be1[:])
            be2_t = cst.tile([64, 2], F32)
            nc.sync.dma_start(be2_t[:], d_be2[:])
            be2h_t = cst.tile([64, 1], F32)
            nc.vector.tensor_scalar_mul(be2h_t[:], be2_t[:, 1:2], 0.5)
            bdec_t = cst.tile([128, 38], F32)
            nc.sync.dma_start(bdec_t[:], d_bdec[:])
            br1_t = cst.tile([128, 2], F32)
            nc.sync.dma_start(br1_t[:], d_br1[:])
            wr2_t = cst.tile([128, 2, 38], F32)
            nc.sync.dma_start(wr2_t[:], d_wr2[:])
            br2_t = cst.tile([38, 1], F32)
            nc.sync.dma_start(br2_t[:], d_br2[:])
            epsT_t = cst.tile([PLAT, bc + 8], F32)
            nc.sync.dma_start(epsT_t[:], d_epsT[:])
            wdec_t = cst.tile([PLAT, 38, 128], F32)
            nc.sync.dma_start(wdec_t[:], d_wdec[:])

            # hfT assembly buffer: [128, half, i(19), NV]
            hfT = hbuf.tile([128, 2, J, NV], F32)

            def process_tile(tinfo, cg0):
                t, g0, G = tinfo
                Rt = G * J
                r0 = g0 * J
                bo = bo_t[:Rt, :Rt]

                # --- input DMAs ---
                ft = io.tile([RF, H], F32, tag="feat_in")
                nc.sync.dma_start(ft[:Rt, :], d_feat[r0:r0 + Rt, :])
                ptst = io.tile([RF, 2], F32, tag="pts_in")
                nc.sync.dma_start(ptst[:Rt, :], d_pts[r0:r0 + Rt, :])
                kvt = io.tile([RF, 8], F32, tag="kv_in")
                nc.sync.dma_start(kvt[:Rt, :], d_kvrem[r0:r0 + Rt, :])
                rit = io.tile([RF, 1], F32, tag="ri_in")
                nc.sync.dma_start(rit[:Rt, :], d_rinv[r0:r0 + Rt, :])

                # --- norms: nf = |F|^2 rows, npts = |p|^2 rows ---
                sqsc = sbB.tile([RF, H], F32, tag="sqscratch")
                nf = small.tile([RF, 1], F32, tag="nf")
                nc.scalar.activation(sqsc[:Rt, :], ft[:Rt, :], AF.Square,
                                     accum_out=nf[:Rt, :])
                np_ = small.tile([RF, 1], F32, tag="npts")
                nc.scalar.activation(sqsc[:Rt, 0:2], ptst[:Rt, :], AF.Square,
                                     accum_out=np_[:Rt, :])

                # --- transposes of F (2 chunks), pts, nf, npts ---
                fT = sbA.tile([128, 2, RF], F32, tag="fT")
                for c in range(2):
                    p = pt.tile([128, RF], F32, tag="ptrans")
                    nc.tensor.transpose(p[:, :Rt], ft[:Rt, c * 128:(c + 1) * 128],
                                        ident[:Rt, :Rt])
                    nc.scalar.copy(fT[:, c, :Rt], p[:, :Rt])
                # ptsT_ext [3, Rt]: rows 0:2 = pts^T, row 2 = ones
                ptsT = sbA.tile([3, RF], F32, tag="ptsT")
                nc.vector.memset(ptsT[0:3, :Rt], 1.0)
                p = pt.tile([128, RF], F32, tag="ptrans")
                nc.tensor.transpose(p[:2, :Rt], ptst[:Rt, :], ident[:Rt, :Rt])
                nc.scalar.copy(ptsT[0:2, :Rt], p[:2, :Rt])
                # norm-row pairs built by transposing [Rt, 2] column tiles:
                # cols (0,1)=(1, -nf/2) (2,3)=(-nf/2, 1) (4,5)=(1, -np/2) (6,7)=(-np/2, 1)
                normin = sbA.tile([RF, 8], F32, tag="normin")
                nc.vector.memset(normin[:Rt, :], 1.0)
                nc.vector.tensor_scalar_mul(normin[:Rt, 1:2], nf[:Rt, :], -0.5)
                nc.vector.tensor_scalar_mul(normin[:Rt, 2:3], nf[:Rt, :], -0.5)
                nc.vector.tensor_scalar_mul(normin[:Rt, 5:6], np_[:Rt, :], -0.5)
                nc.vector.tensor_scalar_mul(normin[:Rt, 6:7], np_[:Rt, :], -0.5)
                nrows = sbA.tile([2, 4, RF], F32, tag="nrows")
                for jj in range(4):
                    p = pt.tile([128, RF], F32, tag="ptrans")
                    nc.tensor.transpose(p[:2, :Rt], normin[:Rt, 2 * jj:2 * jj + 2],
                                        ident[:Rt, :Rt])
                    nc.scalar.copy(nrows[:, jj, :Rt], p[:2, :Rt])

                # --- Gram(feat) - 0.5 nf_i - 0.5 nf_j  (= -0.5 * d2f) ---
                gm = pm.tile([128, 512], F32, tag="pbig")
                nc.tensor.matmul(gm[:Rt, :Rt], fT[:, 0, :Rt], fT[:, 0, :Rt],
                                 start=True, stop=False)
                nc.tensor.matmul(gm[:Rt, :Rt], fT[:, 1, :Rt], fT[:, 1, :Rt],
                                 start=False, stop=False)
                nc.tensor.matmul(gm[:Rt, :Rt], nrows[:, 0, :Rt], nrows[:, 1, :Rt],
                                 start=False, stop=True)
                gmin = sbB.tile([RF, RF], F32, tag="gmin")
                nc.vector.tensor_scalar(gmin[:Rt, :Rt], gm[:Rt, :Rt], 0.0, None, OP.min)
                sf = sbB.tile([RF, RF], F32, tag="sf")
                nc.scalar.activation(sf[:Rt, :Rt], gmin[:Rt, :Rt], AF.Sqrt, scale=-2.0)
                # --- Gram(pts) ---
                gp = pm.tile([128, 512], F32, tag="pbig")
                nc.tensor.matmul(gp[:Rt, :Rt], ptsT[0:2, :Rt], ptsT[0:2, :Rt],
                                 start=True, stop=False)
                nc.tensor.matmul(gp[:Rt, :Rt], nrows[:, 2, :Rt], nrows[:, 3, :Rt],
                                 start=False, stop=True)
                nc.vector.tensor_scalar(gmin[:Rt, :Rt], gp[:Rt, :Rt], 0.0, None, OP.min)
                sp = sbB.tile([RF, RF], F32, tag="sp")
                nc.scalar.activation(sp[:Rt, :Rt], gmin[:Rt, :Rt], AF.Sqrt, scale=-2.0)

                # score = BO - (sf + sp)   (on-block: -(df+dp); off-block <= -1e9)
                nc.vector.tensor_tensor(sf[:Rt, :Rt], sf[:Rt, :Rt], sp[:Rt, :Rt], OP.add)
                score = sbB.tile([RF, RF], F32, tag="score")
                nc.vector.tensor_tensor(score[:Rt, :Rt], bo[:Rt, :Rt], sf[:Rt, :Rt],
                                        OP.subtract)

                # --- top-kv mask S ---
                mx = small.tile([RF, 8], F32, tag="mx")
                nc.vector.max(out=mx[:Rt, :], in_=score[:Rt, :Rt])
                done = small.tile([RF, 8], U32, tag="done")
                nc.vector.tensor_scalar(done[:Rt, :], kvt[:Rt, :], 0.0, None, OP.is_le)
                nc.vector.copy_predicated(mx[:Rt, :], done[:Rt, :], negt[:Rt, :])
                rep = sbB.tile([RF, RF], F32, tag="rep")
                nc.vector.match_replace(out=rep[:Rt, :Rt], in_to_replace=mx[:Rt, :],
                                        in_values=score[:Rt, :Rt], imm_value=NEG)
                S = sbA.tile([RF, RF], F32, tag="S")
                nc.vector.tensor_tensor(S[:Rt, :Rt], score[:Rt, :Rt], rep[:Rt, :Rt],
                                        OP.is_gt)

                # --- A matrix: Af [Rt+1, Rt], rows 0:Rt = (diag(Dinv) Araw)^T, last = 1
                SR = sbB.tile([RF, RF + 1], F32, tag="SR")
                nc.vector.tensor_scalar(SR[:Rt, :Rt], S[:Rt, :Rt], rit[:Rt, :], None,
                                        OP.mult)
                nc.vector.memset(SR[:Rt, Rt:Rt + 1], 1.0)
                araw = pm.tile([128, 512], F32, tag="pbig")
                nc.tensor.matmul(araw[:Rt, :Rt + 1], S[:Rt, :Rt], SR[:Rt, :Rt + 1],
                                 start=True, stop=True)
                dinv = small.tile([RF, 1], F32, tag="dinv")
                nc.vector.reciprocal(dinv[:Rt, :], araw[:Rt, Rt:Rt + 1])
                dz = small.tile([RF, 1], U32, tag="dz")
                nc.vector.tensor_scalar(dz[:Rt, :], araw[:Rt, Rt:Rt + 1], 0.0, None,
                                        OP.is_le)
                nc.vector.copy_predicated(dinv[:Rt, :], dz[:Rt, :], zcol[:Rt, :])
                asc = sbB.tile([RF, RF + 1], F32, tag="asc")
                nc.vector.tensor_scalar(asc[:Rt, :Rt], araw[:Rt, :Rt], dinv[:Rt, :],
                                        None, OP.mult)
                nc.vector.memset(asc[:Rt, Rt:Rt + 1], 1.0)
                pA = pt.tile([128, RF], F32, tag="ptrans")
                nc.tensor.transpose(pA[:Rt + 1, :Rt], asc[:Rt, :Rt + 1],
                                    ident[:Rt, :Rt])
                Af = sbA.tile([RF + 1, RF], F32, tag="Af")
                nc.scalar.copy(Af[:Rt + 1, :Rt], pA[:Rt + 1, :Rt])

                if stages < 2:
                    nc.sync.dma_start(d_y[g0:g0 + G, :],
                                      Af[:G, :38])
                    return
                # ---------------- 3 hconv layers ----------------
                h = None
                for l in range(3):
                    xt = pm.tile([128, 512], F32, tag="pbig")
                    if l == 0:
                        nc.tensor.matmul(xt[:Rt, :H], ptsT[:, :Rt], wc0_t[:],
                                         start=True, stop=True)
                    else:
                        hT = sbC.tile([128, 2, RF], F32, tag="hT")
                        for c in range(2):
                            p2 = pt.tile([128, RF], F32, tag="ptrans")
                            nc.tensor.transpose(p2[:, :Rt],
                                                h[:Rt, c * 128:(c + 1) * 128],
                                                ident[:Rt, :Rt])
                            nc.scalar.copy(hT[:, c, :Rt], p2[:, :Rt])
                        wl = l - 1
                        nc.tensor.matmul(xt[:Rt, :H], hT[:, 0, :Rt],
                                         wga_t[:, wl, :], start=True, stop=False)
                        nc.tensor.matmul(xt[:Rt, :H], hT[:, 1, :Rt],
                                         wgb_t[:, wl, :], start=False, stop=True)
                    xts = sbC.tile([RF + 1, H], F32, tag="xts")
                    if l == 0:
                        nc.vector.tensor_copy(xts[:Rt, :], xt[:Rt, :H])
                    else:
                        nc.vector.tensor_tensor(xts[:Rt, :], xt[:Rt, :H],
                                                cgnb_t[:Rt, l - 1, :], OP.add)
                    nc.sync.dma_start(xts[Rt:Rt + 1, :], d_bg3[l:l + 1, :])
                    agg = pm.tile([128, 512], F32, tag="pbig")
                    nc.tensor.matmul(agg[:Rt, :H], Af[:Rt + 1, :Rt], xts[:Rt + 1, :],
                                     start=True, stop=True)
                    # relu + LN stats
                    hr = sbC.tile([RF, H], F32, tag="hrelu")
                    rsum = small.tile([RF, 1], F32, tag="rsum")
                    nc.scalar.activation(hr[:Rt, :], agg[:Rt, :H], AF.Relu,
                                         accum_out=rsum[:Rt, :])
                    ssq = small.tile([RF, 1], F32, tag="ssq")
                    nc.scalar.activation(sqsc[:Rt, :], hr[:Rt, :], AF.Square,
                                         accum_out=ssq[:Rt, :])
                    mu = small.tile([RF, 1], F32, tag="mu")
                    nc.vector.tensor_scalar_mul(mu[:Rt, :], rsum[:Rt, :], 1.0 / H)
                    var = small.tile([RF, 1], F32, tag="var")
                    nc.vector.tensor_scalar_mul(var[:Rt, :], ssq[:Rt, :], 1.0 / H)
                    mu2 = small.tile([RF, 1], F32, tag="mu2")
                    nc.vector.tensor_tensor(mu2[:Rt, :], mu[:Rt, :], mu[:Rt, :], OP.mult)
                    nc.vector.tensor_tensor(var[:Rt, :], var[:Rt, :], mu2[:Rt, :],
                                            OP.subtract)
                    sg = small.tile([RF, 1], F32, tag="sg")
                    nc.scalar.activation(sg[:Rt, :], var[:Rt, :], AF.Sqrt, bias=epsc[:Rt, :])
                    rs = small.tile([RF, 1], F32, tag="rs")
                    nc.vector.reciprocal(rs[:Rt, :], sg[:Rt, :])
                    h = sbC.tile([RF, H], F32, tag=f"h{l}")
                    nc.vector.tensor_scalar(h[:Rt, :], hr[:Rt, :], mu[:Rt, :],
                                            rs[:Rt, :], OP.subtract, OP.mult)

                if stages < 3:
                    nc.sync.dma_start(d_y[g0:g0 + G, :], h[:G, :38])
                    return
                # ---------------- attention ----------------
                hT = sbC.tile([128, 2, RF], F32, tag="hT")
                for c in range(2):
                    p2 = pt.tile([128, RF], F32, tag="ptrans")
                    nc.tensor.transpose(p2[:, :Rt], h[:Rt, c * 128:(c + 1) * 128],
                                        ident[:Rt, :Rt])
                    nc.scalar.copy(hT[:, c, :Rt], p2[:, :Rt])
                qkvs = sbB.tile([RF, 3 * H], F32, tag="qkvs")
                for nh in range(2):
                    qkv = pm.tile([128, 512], F32, tag="pbig")
                    s0, s1 = nh * 384, (nh + 1) * 384
                    nc.tensor.matmul(qkv[:Rt, :384], hT[:, 0, :Rt],
                                     wqa_t[:, s0:s1], start=True, stop=False)
                    nc.tensor.matmul(qkv[:Rt, :384], hT[:, 1, :Rt],
                                     wqb_t[:, s0:s1], start=False, stop=True)
                    nc.vector.tensor_tensor(qkvs[:Rt, s0:s1], qkv[:Rt, :384],
                                            cqb_t[:Rt, s0:s1], OP.add)
                if sub < 1:
                    nc.sync.dma_start(d_y[g0:g0 + G, :], qkvs[:G, :38])
                    return
                # q,k transposed per head-pair; q scaled by 1/8
                qT = sbA.tile([64, 4, RF], F32, tag="qT")
                kT = sbA.tile([64, 4, 128], F32, tag="kT")
                if Rt < 128:
                    nc.vector.memset(kT[:, :, Rt:128], 0.0)
                for hh in range(4):
                    p2 = pt.tile([128, RF], F32, tag="ptrans")
                    nc.tensor.transpose(p2[:64, :Rt], qkvs[:Rt, hh * 64:(hh + 1) * 64],
                                        ident[:Rt, :Rt])
                    nc.scalar.mul(qT[:, hh, :Rt], p2[:64, :Rt], 0.125)
                    p2 = pt.tile([128, RF], F32, tag="ptrans")
                    nc.tensor.transpose(p2[:64, :Rt],
                                        qkvs[:Rt, H + hh * 64:H + (hh + 1) * 64],
                                        ident[:Rt, :Rt])
                    nc.scalar.copy(kT[:, hh, :Rt], p2[:64, :Rt])
                if sub < 2:
                    nc.sync.dma_start(d_y[g0:g0 + G, :], qT[:G, 0, :38])
                    return
                # scores for 4 heads into one psum [Rt, 4*Rt]
                scs = []
                for hh in range(4):
                    sc = pm.tile([128, 512], F32, tag="pbig", name=f"sc{hh}")
                    nc.tensor.matmul(sc[:Rt, :128], qT[:, hh, :Rt], kT[:, hh, :],
                                     start=True, stop=True)
                    scs.append(sc)
                if sub == 2.01:
                    tmpo = sbB.tile([RF, 4 * RF], F32, tag="tmpo")
                    nc.vector.tensor_copy(tmpo[:Rt, :4 * Rt], sc[:Rt, :4 * Rt])
                    nc.sync.dma_start(d_y[g0:g0 + G, :], tmpo[:G, :38])
                    return
                pexp = sbB.tile([RF, 512], F32, tag="pexp")
                for hh in range(4):
                    nc.scalar.activation(pexp[:Rt, hh * 128:(hh + 1) * 128],
                                         scs[hh][:Rt, :128], AF.Exp)
                # mask off-block + per-head row sums
                if sub == 2.02:
                    nc.sync.dma_start(d_y[g0:g0 + G, :], pexp[:G, :38])
                    return
                sums = small.tile([RF, 4], F32, tag="sums")
                pm4 = sbB.tile([RF, 512], F32, tag="pm4")
                nc.vector.tensor_tensor(
                    pm4[:Rt, :].rearrange("p (h j) -> p h j", h=4),
                    pexp[:Rt, :].rearrange("p (h j) -> p h j", h=4),
                    bo4_t[:Rt, :].rearrange("p (h j) -> p h j", h=4), OP.mult)
                nc.vector.tensor_reduce(
                    sums[:Rt, :],
                    pm4[:Rt, :].rearrange("p (h j) -> p h j", h=4),
                    axis=mybir.AxisListType.X, op=OP.add)
                if sub == 2.03:
                    nc.sync.dma_start(d_y[g0:g0 + G, :], pm4[:G, :38])
                    return
                rsum4 = small.tile([RF, 4], F32, tag="rsum4")
                nc.vector.reciprocal(rsum4[:Rt, :], sums[:Rt, :])
                if sub == 2.04:
                    nc.sync.dma_start(d_y[g0:g0 + G, :], sums[:G, :4])
                    return
                att = sbB.tile([RF, 512], F32, tag="att")
                nc.vector.tensor_tensor(
                    att[:Rt, :].rearrange("p (h j) -> p h j", h=4),
                    pm4[:Rt, :].rearrange("p (h j) -> p h j", h=4),
                    rsum4[:Rt, :, None].to_broadcast((Rt, 4, 128)),
                    OP.mult)
                if sub < 3:
                    nc.sync.dma_start(d_y[g0:g0 + G, :], att[:G, :38])
                    return
                # attT per head, then AV; oT4 [64, 4, RF]
                oT = sbA.tile([64, 4, RF], F32, tag="oT")
                for hh in range(4):
                    pa = pt.tile([128, RF], F32, tag="ptrans")
                    nc.tensor.transpose(pa[:Rt, :Rt],
                                        att[:Rt, hh * 128:hh * 128 + Rt],
                                        ident[:Rt, :Rt])
                    aT = sbB.tile([RF, RF], F32, tag="aT")
                    nc.scalar.copy(aT[:Rt, :Rt], pa[:Rt, :Rt])
                    po = pm.tile([128, 512], F32, tag="pbig", name=f"po{hh}")
                    nc.tensor.matmul(po[:64, :Rt], qkvs[:Rt, 512 + hh * 64:512 + (hh + 1) * 64],
                                     aT[:Rt, :Rt], start=True, stop=True)
                    nc.scalar.copy(oT[:, hh, :Rt], po[:64, :Rt])
                if sub < 4:
                    nc.sync.dma_start(d_y[g0:g0 + G, :], oT[:G, 0, :38])
                    return
                hat = pm.tile([128, 512], F32, tag="pbig")
                for hh in range(4):
                    nc.tensor.matmul(hat[:Rt, :H], oT[:, hh, :Rt], wao4_t[:, hh, :],
                                     start=(hh == 0), stop=(hh == 3))
                hats = sbC.tile([RF, H], F32, tag="hats")
                nc.vector.tensor_tensor(hats[:Rt, :], hat[:Rt, :H],
                                        caob_t[:Rt, :], OP.add)
                # scatter transposed into hfT
                col0 = g0 - cg0
                for c in range(2):
                    p2 = pt.tile([128, RF], F32, tag="ptrans")
                    nc.tensor.transpose(p2[:, :Rt], hats[:Rt, c * 128:(c + 1) * 128],
                                        ident[:Rt, :Rt])
                    dst = hfT[:, c, :, col0:col0 + G].rearrange("p i g -> p g i")
                    nc.vector.tensor_copy(dst, p2[:, :Rt].rearrange(
                        "p (g i) -> p g i", i=J))

            def vae(cg0, cg1):
                GC = cg1 - cg0
                if GC < NV:
                    nc.vector.memset(hfT[:, :, :, GC:NV], 0.0)
                # e1: out e1r [2][128, NV]; stream we1 per-kk
                pse = [pr.tile([128, 512], F32, tag=f"pr1_{m}", name=f"pse{m}")
                       for m in range(2)]
                for kk in range(38):
                    i, half = kk // 2, kk % 2
                    wkt = wk.tile([128, H], F32, tag="wk1")
                    nc.sync.dma_start(wkt[:], d_we1[:, kk, :])
                    for m in range(2):
                        nc.tensor.matmul(pse[m][:, :NV], wkt[:, m * 128:(m + 1) * 128],
                                         hfT[:, half, i, :], start=(kk == 0),
                                         stop=(kk == 37))
                e1r = []
                for m in range(2):
                    r = sbB.tile([128, NV], F32, tag=f"e1r{m}", name=f"e1r{m}")
                    nc.scalar.activation(r[:], pse[m][:, :NV], AF.Relu,
                                         bias=be1_t[:, m:m + 1])
                    e1r.append(r)
                # e2 -> mu, lv psums [64, NV] each
                pmu = pm.tile([128, 512], F32, tag="pbig", name="pmu")
                nc.tensor.matmul(pmu[:PLAT, :NV], we2_t[:, 0, 0, :], e1r[0][:],
                                 start=True, stop=False)
                nc.tensor.matmul(pmu[:PLAT, :NV], we2_t[:, 1, 0, :], e1r[1][:],
                                 start=False, stop=True)
                plv = pm.tile([128, 512], F32, tag="pbig", name="plv")
                nc.tensor.matmul(plv[:PLAT, :NV], we2_t[:, 0, 1, :], e1r[0][:],
                                 start=True, stop=False)
                nc.tensor.matmul(plv[:PLAT, :NV], we2_t[:, 1, 1, :], e1r[1][:],
                                 start=False, stop=True)
                mus = sbB.tile([PLAT, NV], F32, tag="mus")
                nc.scalar.activation(mus[:], pmu[:PLAT, :NV], AF.Identity,
                                     bias=be2_t[:, 0:1])
                # exp(0.5*(lv + b)) = Exp(psum*0.5 + 0.5*b)
                ex = sbB.tile([PLAT, NV], F32, tag="ex")
                nc.scalar.activation(ex[:], plv[:PLAT, :NV], AF.Exp, scale=0.5,
                                     bias=be2h_t[:, 0:1])
                nc.vector.tensor_tensor(ex[:], ex[:], epsT_t[:, cg0:cg0 + NV], OP.mult)
                zT = sbB.tile([PLAT, NV], F32, tag="zT")
                nc.vector.tensor_tensor(zT[:], ex[:], mus[:], OP.add)
                # dec + r1 accumulation
                psr = [pr.tile([128, 512], F32, tag=f"pr1_{m}", name=f"pr1_{m}") for m in range(2)]
                for kk in range(38):
                    ph = pm.tile([128, 512], F32, tag="pbig")
                    nc.tensor.matmul(ph[:, :NV], wdec_t[:, kk, :], zT[:],
                                     start=True, stop=True)
                    hrr = sbC.tile([128, NV], F32, tag="hrr")
                    nc.scalar.activation(hrr[:], ph[:, :NV], AF.Identity,
                                         bias=bdec_t[:, kk:kk + 1])
                    wkt = wk.tile([128, H], F32, tag="wk2")
                    nc.sync.dma_start(wkt[:], d_wr1[:, kk, :])
                    for m in range(2):
                        nc.tensor.matmul(psr[m][:, :NV], wkt[:, m * 128:(m + 1) * 128],
                                         hrr[:], start=(kk == 0), stop=(kk == 37))
                r1r = []
                for m in range(2):
                    r = sbB.tile([128, NV], F32, tag=f"r1r{m}")
                    nc.scalar.activation(r[:], psr[m][:, :NV], AF.Relu,
                                         bias=br1_t[:, m:m + 1])
                    r1r.append(r)
                ps = pm.tile([128, 512], F32, tag="pbig")
                nc.tensor.matmul(ps[:38, :NV], wr2_t[:, 0, :], r1r[0][:],
                                 start=True, stop=False)
                nc.tensor.matmul(ps[:38, :NV], wr2_t[:, 1, :], r1r[1][:],
                                 start=False, stop=True)
                predT = sbB.tile([38, NV], F32, tag="predT")
                nc.scalar.activation(predT[:], ps[:38, :NV], AF.Identity, bias=br2_t[:])
                # transpose out and DMA
                for off in range(0, GC, 128):
                    w = min(128, GC - off)
                    p2 = pt.tile([128, RF], F32, tag="ptrans")
                    nc.tensor.transpose(p2[:w, :38], predT[:, off:off + w],
                                        ident[:38, :38])
                    ob = sbC.tile([128, 38], F32, tag="ob")
                    nc.scalar.copy(ob[:w, :], p2[:w, :38])
                    nc.sync.dma_start(d_y[cg0 + off:cg0 + off + w, :], ob[:w, :])

            for (cg0, cg1, tl) in chunks:
                for tinfo in tl:
                    process_tile(tinfo, cg0)
                if stages < 4:
                    if stages == 3 and sub >= 5:
                        nc.sync.dma_start(
                            d_y[cg0:cg1, :],
                            hfT[:38, 0, 0, 0:cg1 - cg0].rearrange("p g -> g p"))
                    continue
                vae(cg0, cg1)

    nc.finalize()
    return nc


def _host_prep(inputs, bc=BC):
    """Returns (shared weight arrays dict, per-core input dicts list)."""
    f32 = np.float32
    w_init = inputs['w_init'].astype(f32)
    b_init = inputs['b_init'].astype(f32)
    w_gnn = inputs['w_gnn'].astype(f32)
    b_gnn = inputs['b_gnn'].astype(f32)
    ln_g = inputs['ln_g'].astype(f32)
    ln_b = inputs['ln_b'].astype(f32)
    w_qkv = inputs['w_qkv'].astype(f32)
    b_qkv = inputs['b_qkv'].astype(f32)
    w_ao = inputs['w_ao'].astype(f32)
    b_ao = inputs['b_ao'].astype(f32)

    sh = {}
    # layer0: xt1 = [pts|1] @ wc0, wc0 = [w_init^T; b_init] @ W0^T
    wc0 = np.concatenate([w_init.T, b_init[None, :]], 0) @ w_gnn[0].T
    sh['wc0'] = np.ascontiguousarray(wc0, f32)
    # layers 1,2: W~^T = diag(g_{l-1}) W_l^T ; c_l = W_l @ beta_{l-1}
    wga = np.zeros((128, 2, H), f32)
    wgb = np.zeros((128, 2, H), f32)
    cgn = np.zeros((2, H), f32)
    for l in (1, 2):
        wt = (ln_g[l - 1][:, None] * w_gnn[l].T)  # [256(c), 256(o)]
        cgn[l - 1] = w_gnn[l] @ ln_b[l - 1]       # [256]
        wga[:, l - 1, :] = wt[0:128]
        wgb[:, l - 1, :] = wt[128:256]
    sh['wga'] = wga
    sh['wgb'] = wgb
    sh['cgn'] = cgn
    sh['bg3'] = np.ascontiguousarray(b_gnn, f32)
    wq = (ln_g[2][:, None] * w_qkv.T)             # [256, 768]
    cq = w_qkv @ ln_b[2] + b_qkv
    sh['wqa'] = np.ascontiguousarray(wq[0:128], f32)
    sh['wqb'] = np.ascontiguousarray(wq[128:256], f32)
    sh['cq'] = np.ascontiguousarray(cq[None, :], f32)
    sh['wao4'] = np.ascontiguousarray(
        w_ao.T.reshape(4, 64, H).transpose(1, 0, 2), f32)
    sh['cao'] = np.ascontiguousarray(b_ao[None, :], f32)
    # VAE weights
    we1 = inputs['w_e1'].astype(f32)     # [256, 4864]
    sh['we1'] = np.ascontiguousarray(
        we1.T.reshape(38, 128, H).transpose(1, 0, 2), f32)
    sh['be1'] = np.ascontiguousarray(inputs['b_e1'].astype(f32).reshape(2, 128).T)
    we2 = inputs['w_e2'].astype(f32)     # [128, 256]
    # [c(128), half, m2(mu/lv), 64]
    sh['we2'] = np.ascontiguousarray(
        we2.T.reshape(2, 128, 2, 64).transpose(1, 0, 2, 3), f32)
    sh['be2'] = np.ascontiguousarray(
        inputs['b_e2'].astype(f32).reshape(2, 64).T)
    wdec = inputs['w_dec'].astype(f32)   # [4864, 64]
    sh['wdec'] = np.ascontiguousarray(
        wdec.reshape(38, 128, PLAT).transpose(2, 0, 1), f32)
    sh['bdec'] = np.ascontiguousarray(
        inputs['b_dec'].astype(f32).reshape(38, 128).T, f32)
    wr1 = inputs['w_r1'].astype(f32)
    sh['wr1'] = np.ascontiguousarray(
        wr1.T.reshape(38, 128, H).transpose(1, 0, 2), f32)
    sh['br1'] = np.ascontiguousarray(inputs['b_r1'].astype(f32).reshape(2, 128).T)
    wr2 = inputs['w_r2'].astype(f32)     # [38, 256]
    sh['wr2'] = np.ascontiguousarray(
        wr2.T.reshape(2, 128, 38).transpose(1, 0, 2), f32)
    sh['br2'] = inputs['b_r2'].astype(f32).reshape(38, 1)
    # block-diag masks
    bo = np.full((RF, RF), NEG, f32)
    for g in range(GPT):
        bo[g * J:(g + 1) * J, g * J:(g + 1) * J] = 0.0
    sh['bo'] = bo
    bo4 = np.zeros((RF, 512), f32)
    for hh in range(4):
        bo4[:, hh * 128:hh * 128 + RF] = (bo == 0.0)
    sh['bo4'] = bo4

    pts = inputs['points'].astype(f32)
    feat = inputs['img_features'].astype(f32)
    kv = inputs['k_vals']
    eps = inputs['eps'].astype(f32)
    Ba = pts.shape[0]
    kvrem = (kv.astype(f32).reshape(Ba * J, 1)
             - np.arange(8, dtype=f32)[None, :])
    rinv = (1.0 / kv.astype(f32)).reshape(Ba * J, 1)

    per_core = []
    for c in range(Ba // bc):
        g0, g1 = c * bc, (c + 1) * bc
        r0, r1 = g0 * J, g1 * J
        epsT = np.zeros((PLAT, bc + 8), f32)
        epsT[:, :bc] = eps[g0:g1].T
        m = dict(sh)
        m['pts'] = np.ascontiguousarray(pts.reshape(Ba * J, 2)[r0:r1])
        m['feat'] = np.ascontiguousarray(feat.reshape(Ba * J, H)[r0:r1])
        m['kvrem'] = np.ascontiguousarray(kvrem[r0:r1])
        m['rinv'] = np.ascontiguousarray(rinv[r0:r1])
        m['epsT'] = epsT
        per_core.append(m)
    return per_core


def kernel(**inputs):
    key = 'nc'
    if key not in _CACHE:
        _CACHE[key] = build_nc(BC)
    nc = _CACHE[key]
    in_maps = _host_prep(inputs, BC)
    res = run_bass_kernel_spmd(nc, in_maps, core_ids=list(range(NCORES)))
    ys = [res.results[c]['y'] for c in range(NCORES)]
    out = np.concatenate(ys, 0).reshape(B, J, 2)
    return out.astype(np.float32)

